# revision 3
# baseline (speedup 1.0000x reference)
"""Trainium2 Bass kernel for AttentionFlowLayer scores.

S[b,t,j] = C[b,t,:]@wC + Q[b,j,:]@wQ + sum_d C[b,t,d]*wCmQ[d]*Q[b,j,d] + bias

Full shapes: C [64,2048,128] f32, Q [64,512,128] f32 -> S [64,2048,512] f32.
Data-parallel over batch across 8 NeuronCores (8 batches per core).

Per core, per batch b:
  - DMA C[b] -> SBUF natural layout [t%128, 16*128] and Q[b] -> [j%128, 4*128].
  - PE-transpose C/Q 128x128 tiles to put d on partitions (fp32 transpose
    is exact).
  - Main matmul in float32r "split" form: C^T rounded to f32r (CT_h) plus
    residual (CT_l); rhs QW^T = (Q^T * wCmQ) likewise split into QWT_h/QWT_l.
    S_psum = CT_h.T@QWT_h + CT_l.T@QWT_h + CT_h.T@QWT_l  (3 f32r matmuls at
    1 cyc/row vs fp32's 4; dropped CT_l.T@QWT_l term is ~2^-24 relative).
  - p1 = C@wC via fp32 matmul on CT_h+CT_l (N=1, exact); p2 = Q@wQ replicated
    across partitions via one fp32 matmul with (wQ outer ones) weights; bias
    folded into the p2 copy.
  - Epilogue: out = S_psum + p1[t] + (p2[j]+bias), split between ACT
    (activation bias path) and DVE (scalar_tensor_tensor) for balance.
"""

import os
import sys

for _p in ("/opt/trn_rl_repo", "/opt/pypackages"):
    if _p not in sys.path and os.path.isdir(_p):
        sys.path.append(_p)

import numpy as np

import concourse.bass as bass
import concourse.mybir as mybir
import concourse.tile as tile
from concourse import bacc
from concourse.bass import ds, ts
from concourse.bass_utils import run_bass_kernel_spmd
from concourse.masks import make_identity

F32 = mybir.dt.float32
F32R = mybir.dt.float32r
AF = mybir.ActivationFunctionType
ALU = mybir.AluOpType

N_CORES = 8
B_FULL, T, D = 64, 2048, 128
J = 512
B_LOC = B_FULL // N_CORES  # 8 batches per core
N_TTILE = T // 128  # 16

# Precision mode: "r3" = split f32r (near-fp32 exact), "r1" = single f32r
# pass (~2e-4 rel err), "r2" = drop rhs-residual term (~1.4e-4).
MODE = os.environ.get("KERNEL_MODE", "r3")
# Fraction (out of 16 tiles) routed through the ACT epilogue path.
ACT_EPI = int(os.environ.get("KERNEL_ACT_EPI", "10"))


def _build_nc():
    nc = bacc.Bacc("TRN2", target_bir_lowering=False, debug=False,
                   num_devices=N_CORES)
    C_d = nc.dram_tensor("C_s", [B_LOC, T, D], F32, kind="ExternalInput")
    Q_d = nc.dram_tensor("Q_s", [B_LOC, J, D], F32, kind="ExternalInput")
    wc_d = nc.dram_tensor("wc_col", [128, 1], F32, kind="ExternalInput")
    wcmq_d = nc.dram_tensor("wcmq_col", [128, 1], F32, kind="ExternalInput")
    bias_d = nc.dram_tensor("bias_rep", [128, 1], F32, kind="ExternalInput")
    wqo_d = nc.dram_tensor("wq_ones", [128, 128], F32, kind="ExternalInput")
    S_d = nc.dram_tensor("S_s", [B_LOC, T, J], F32, kind="ExternalOutput")

    r3 = MODE == "r3"
    r2 = MODE == "r2"

    with tile.TileContext(nc) as tc:
        with tc.tile_pool(name="const", bufs=1) as const_pool, \
             tc.tile_pool(name="cnat", bufs=2) as cnat_pool, \
             tc.tile_pool(name="qside", bufs=2) as qside_pool, \
             tc.tile_pool(name="ct", bufs=3) as ct_pool, \
             tc.tile_pool(name="small", bufs=4) as small_pool, \
             tc.tile_pool(name="outsb", bufs=4) as out_pool, \
             tc.tile_pool(name="ps_tr", bufs=2, space="PSUM") as ps_tr, \
             tc.tile_pool(name="ps_s", bufs=2, space="PSUM") as ps_s, \
             tc.tile_pool(name="ps_p1", bufs=2, space="PSUM") as ps_p1, \
             tc.tile_pool(name="ps_p2", bufs=2, space="PSUM") as ps_p2:

            ident = const_pool.tile([128, 128], F32, name="ident")
            make_identity(nc, ident[:])
            wc_sb = const_pool.tile([128, 1], F32, name="wc_sb")
            nc.sync.dma_start(wc_sb[:], wc_d.ap())
            wcmq_sb = const_pool.tile([128, 1], F32, name="wcmq_sb")
            nc.sync.dma_start(wcmq_sb[:], wcmq_d.ap())
            bias_sb = const_pool.tile([128, 1], F32, name="bias_sb")
            nc.sync.dma_start(bias_sb[:], bias_d.ap())
            wqo_sb = const_pool.tile([128, 128], F32, name="wqo_sb")
            nc.sync.dma_start(wqo_sb[:], wqo_d.ap())

            C_ap = C_d.ap()
            Q_ap = Q_d.ap()
            S_ap = S_d.ap()

            for b in range(B_LOC):
                c_nat = cnat_pool.tile([128, T], F32, name="c_nat", tag="c_nat")
                nc.sync.dma_start(
                    c_nat[:].rearrange("p (n d) -> p n d", d=D),
                    C_ap[b].rearrange("(n p) d -> p n d", p=128))
                q_nat = qside_pool.tile([128, J], F32, name="q_nat", tag="q_nat")
                nc.sync.dma_start(
                    q_nat[:].rearrange("p (n d) -> p n d", d=D),
                    Q_ap[b].rearrange("(n p) d -> p n d", p=128))

                # Q^T (d on partitions), then QW^T = Q^T * wCmQ split h/l.
                qt = qside_pool.tile([128, J], F32, name="qt", tag="qt")
                for qi in range(J // 128):
                    trq = ps_tr.tile([128, 128], F32, name="trq", tag="tr")
                    nc.tensor.transpose(trq[:], q_nat[:, ts(qi, 128)], ident[:])
                    nc.scalar.activation(qt[:, ts(qi, 128)], trq[:], AF.Copy)
                qwt = qside_pool.tile([128, J], F32, name="qwt", tag="qwt")
                nc.vector.tensor_scalar_mul(qwt[:], qt[:], wcmq_sb[:])
                qwt_h = qside_pool.tile([128, J], F32R, name="qwt_h", tag="qwt_h")
                nc.gpsimd.tensor_copy(qwt_h[:], qwt[:])
                if r3:
                    qwt_l = qside_pool.tile([128, J], F32R, name="qwt_l",
                                            tag="qwt_l")
                    nc.vector.tensor_sub(qwt_l[:], qwt[:], qwt_h[:].bitcast(F32))

                # p2 (replicated over partitions) + bias.
                p2ps = ps_p2.tile([128, J], F32, name="p2ps", tag="p2ps")
                nc.tensor.matmul(p2ps[:], wqo_sb[:], qt[:], start=True, stop=True)
                p2b = qside_pool.tile([128, J], F32, name="p2b", tag="p2b")
                nc.vector.tensor_scalar_add(p2b[:], p2ps[:], bias_sb[:])

                for i in range(N_TTILE):
                    trc = ps_tr.tile([128, 128], F32, name="trc", tag="tr")
                    nc.tensor.transpose(trc[:], c_nat[:, ts(i, 128)], ident[:])
                    ct_h = ct_pool.tile([128, 128], F32R, name="ct_h", tag="ct_h")
                    nc.scalar.activation(ct_h[:], trc[:], AF.Copy)
                    if r3 or r2:
                        ct_l = ct_pool.tile([128, 128], F32R, name="ct_l",
                                            tag="ct_l")
                        nc.vector.tensor_sub(ct_l[:], trc[:],
                                             ct_h[:].bitcast(F32))

                    # p1 column for this t-tile (fp32, exact on CT_h+CT_l).
                    p1ps = ps_p1.tile([128, 1], F32, name="p1ps", tag="p1ps")
                    nc.tensor.matmul(p1ps[:], ct_h[:].bitcast(F32), wc_sb[:],
                                     start=True, stop=not (r3 or r2))
                    if r3 or r2:
                        nc.tensor.matmul(p1ps[:], ct_l[:].bitcast(F32),
                                         wc_sb[:], start=False, stop=True)
                    p1col = small_pool.tile([128, 1], F32, name="p1col",
                                            tag="p1col")
                    nc.vector.tensor_copy(p1col[:], p1ps[:])

                    sps = ps_s.tile([128, J], F32, name="sps", tag="sps")
                    if r3:
                        nc.tensor.matmul(sps[:], ct_h[:], qwt_h[:],
                                         start=True, stop=False)
                        nc.tensor.matmul(sps[:], ct_l[:], qwt_h[:],
                                         start=False, stop=False)
                        nc.tensor.matmul(sps[:], ct_h[:], qwt_l[:],
                                         start=False, stop=True)
                    elif r2:
                        nc.tensor.matmul(sps[:], ct_h[:], qwt_h[:],
                                         start=True, stop=False)
                        nc.tensor.matmul(sps[:], ct_l[:], qwt_h[:],
                                         start=False, stop=True)
                    else:
                        nc.tensor.matmul(sps[:], ct_h[:], qwt_h[:],
                                         start=True, stop=True)

                    out_sb = out_pool.tile([128, J], F32, name="out_sb",
                                           tag="out_sb")
                    if i < ACT_EPI:
                        tmp = out_pool.tile([128, J], F32, name="tmp", tag="tmp")
                        nc.scalar.activation(tmp[:], sps[:], AF.Identity,
                                             bias=p1col[:])
                        nc.vector.tensor_add(out_sb[:], tmp[:], p2b[:])
                    else:
                        nc.vector.scalar_tensor_tensor(
                            out_sb[:], sps[:], p1col[:], p2b[:],
                            ALU.add, ALU.add)
                    nc.sync.dma_start(S_ap[b, ds(i * 128, 128), :], out_sb[:])

    nc.compile()
    return nc


_NC_CACHE = None


def _get_nc():
    global _NC_CACHE
    if _NC_CACHE is None:
        _NC_CACHE = _build_nc()
    return _NC_CACHE


def _make_in_maps(C, Q, weight_C, weight_Q, weight_CmQ, bias):
    C = np.ascontiguousarray(np.asarray(C, dtype=np.float32))
    Q = np.ascontiguousarray(np.asarray(Q, dtype=np.float32))
    wc = np.asarray(weight_C, dtype=np.float32).reshape(128, 1)
    wq = np.asarray(weight_Q, dtype=np.float32).reshape(128, 1)
    wcmq = np.asarray(weight_CmQ, dtype=np.float32).reshape(128, 1)
    bias_rep = np.full((128, 1), float(np.asarray(bias).reshape(-1)[0]),
                       dtype=np.float32)
    wq_ones = np.ascontiguousarray(np.tile(wq, (1, 128)))
    in_maps = []
    for k in range(N_CORES):
        in_maps.append({
            "C_s": np.ascontiguousarray(C[k * B_LOC:(k + 1) * B_LOC]),
            "Q_s": np.ascontiguousarray(Q[k * B_LOC:(k + 1) * B_LOC]),
            "wc_col": wc,
            "wcmq_col": wcmq,
            "bias_rep": bias_rep,
            "wq_ones": wq_ones,
        })
    return in_maps


def _run(in_maps, **kw):
    nc = _get_nc()
    return run_bass_kernel_spmd(nc, in_maps, core_ids=list(range(N_CORES)), **kw)


def kernel(C, Q, weight_C, weight_Q, weight_CmQ, bias):
    in_maps = _make_in_maps(C, Q, weight_C, weight_Q, weight_CmQ, bias)
    res = _run(in_maps)
    return np.concatenate([r["S_s"] for r in res.results], axis=0)


def _install_ntff_hook():
    """Provide antenv.axon_hooks (absent on this image) backed by the
    libaxon_pjrt.so NRT-profile C ABI, so trace=True works under axon."""
    import types
    if "antenv.axon_hooks" in sys.modules:
        return
    try:
        from trn_agent_boot.trn_boot import _ntff_profile_via_ctypes
        hook = _ntff_profile_via_ctypes("/opt/axon/libaxon_pjrt.so")
    except Exception:
        hook = None
    mod = types.ModuleType("antenv.axon_hooks")
    _state = {"hook": hook}
    mod.set_axon_ntff_profile_hook = lambda h: _state.__setitem__("hook", h)
    mod.get_axon_ntff_profile_hook = lambda: _state["hook"]
    sys.modules["antenv.axon_hooks"] = mod


def kernel_traced(C, Q, weight_C, weight_Q, weight_CmQ, bias, **kw):
    """Like kernel() but with NTFF tracing; returns (out, BassKernelResults)."""
    _install_ntff_hook()
    in_maps = _make_in_maps(C, Q, weight_C, weight_Q, weight_CmQ, bias)
    res = _run(in_maps, trace=True, **kw)
    out = np.concatenate([r["S_s"] for r in res.results], axis=0)
    return out, res
